# revision 14
# baseline (speedup 1.0000x reference)
"""Trainium2 Bass kernel for nn_CrossAtten: cross-attention
out = softmax((q Wq^T)(kv Wk^T)^T / sqrt(D)) @ (kv Wv^T) @ Wout^T + bout

Shapes (hardcoded): q,kv [4,16,2048,128] fp32; Wq,Wout [128,128]; Wkv [256,128]; bout [128].
Sharding: batch*heads (64 pairs) split 8 per NeuronCore across 8 cores (pure data parallel).

v2 design (bf16 pipeline, jt-major, packed denominator):
  A    = Wq^T @ Wk          -> scores S^T = (A^T q^T)^T kv^T  (u = qA)
  WvoT = Wv^T @ Wout^T      -> PV matmul directly yields final projection
  bout folded into vproj (vproj += bout) so pv accumulates pv + bout*dn and
  the final divide by dn yields out + bout directly.
  All PE operands bf16 (1 cyc/row, FWL weight loads); PSUM accum stays fp32.
  Main loop is jt-major over i-halves (1024 i per half): the kvT[jt]/vproj[jt]
  stationaries are shared across the half's 2 chunks, exp runs as one
  FD=1024 ACTIVATE per (half, jt), and the two denominator matmuls (M=1)
  are packed into one PSUM bank at col strips 0/32 via tile_position so they
  stream concurrently. dn accumulates with start=False onto a memset-zeroed
  bank (DVE memset leaves has_written bits, so matmuls accumulate onto 0).
"""
import sys

if "/opt/trn_rl_repo" not in sys.path:
    sys.path.insert(0, "/opt/trn_rl_repo")

from contextlib import ExitStack

import numpy as np

import concourse.bacc as bacc
import concourse.tile as tile
import concourse.mybir as mybir
from concourse.bass_utils import run_bass_kernel_spmd

B, H, I, J, D = 4, 16, 2048, 2048, 128
BH = B * H
N_CORES = 8
PER_CORE = BH // N_CORES          # 8 (b,h) pairs per core
P = 128                           # partitions
IT = I // P                       # 16 i-tiles
JT = J // P                       # 16 j-tiles
IH = 1024                         # i-half width
IC = 512                          # i-chunk (columns per scores/PV matmul)
SCALE = D ** -0.5

F32 = mybir.dt.float32
BF16 = mybir.dt.bfloat16
EXP = mybir.ActivationFunctionType.Exp

_cache = {}


def _build(repeat=1):
    nc = bacc.Bacc(
        "TRN2",
        target_bir_lowering=False,
        debug=False,
        enable_asserts=False,
        num_devices=N_CORES,
    )

    q_d = nc.dram_tensor("q", [PER_CORE, I, D], F32, kind="ExternalInput").ap()
    kv_d = nc.dram_tensor("kv", [PER_CORE, J, D], F32, kind="ExternalInput").ap()
    a_d = nc.dram_tensor("A", [D, D], F32, kind="ExternalInput").ap()
    wvo_d = nc.dram_tensor("WvoT", [D, D], F32, kind="ExternalInput").ap()
    boutb_d = nc.dram_tensor("bout_b", [P, 4 * D], F32, kind="ExternalInput").ap()
    ident_d = nc.dram_tensor("ident", [P, P], F32, kind="ExternalInput").ap()
    out_d = nc.dram_tensor("out", [PER_CORE, I, D], F32, kind="ExternalOutput").ap()

    with tile.TileContext(nc) as tc, ExitStack() as ctx:
        const = ctx.enter_context(tc.tile_pool(name="const", bufs=1))
        qkv = ctx.enter_context(tc.tile_pool(name="qkv", bufs=2))
        tp = ctx.enter_context(tc.tile_pool(name="tp", bufs=2))
        ep = ctx.enter_context(tc.tile_pool(name="ep", bufs=3))
        fin = ctx.enter_context(tc.tile_pool(name="fin", bufs=2))
        # PSUM: sc 2x[128,1024]f32 (4 banks) + pv 2x[128,512]f32 (2) +
        # dn 1x[128,512]f32 (1) + scratch 1x[128,512]f32 (1) = 8 banks
        ps_sc = ctx.enter_context(tc.tile_pool(name="ps_sc", bufs=2, space="PSUM"))
        ps_pv = ctx.enter_context(tc.tile_pool(name="ps_pv", bufs=2, space="PSUM"))
        ps_dn = ctx.enter_context(tc.tile_pool(name="ps_dn", bufs=1, space="PSUM"))
        ps_sx = ctx.enter_context(tc.tile_pool(name="ps_sx", bufs=1, space="PSUM"))

        # ---- constants (loaded / prepared once, cast to bf16 in DMA) ----
        ident_bf = const.tile([P, P], BF16, tag="ident_bf")
        nc.gpsimd.dma_start(ident_bf[:], ident_d)
        a_bf = const.tile([D, D], BF16, tag="a_bf")
        nc.gpsimd.dma_start(a_bf[:], a_d)
        wvo_bf = const.tile([D, D], BF16, tag="wvo_bf")
        nc.gpsimd.dma_start(wvo_bf[:], wvo_d)
        # bias broadcast over j-partitions, tiled 4x along free dim so a
        # 4-jt vproj evacuation adds it in one tensor_tensor
        bout_b = const.tile([P, 4 * D], F32, tag="bout_b")
        nc.sync.dma_start(bout_b[:], boutb_d)
        ones_bf = const.tile([P, 1], BF16, tag="ones_bf")
        nc.vector.memset(ones_bf[:], 1.0)
        ones_f32 = const.tile([P, 1], F32, tag="ones_f32")
        nc.vector.memset(ones_f32[:], 1.0)

        tasks = [(r, b) for r in range(repeat) for b in range(PER_CORE)]
        TILES = {}

        def _loads(k):
            bh = tasks[k][1]
            # partition p holds rows 16p..16p+15 (i = 16p + r): contiguous
            # 2KB fp32 per partition per 4-row group; SWDGE casts to bf16.
            kv_sb = qkv.tile([P, J], BF16, tag="kv_sb", name=f"kv_sb_{k}")
            kvv = kv_d[bh].rearrange("(p r) d -> p r d", r=JT)
            kvs = kv_sb[:].rearrange("p (r d) -> p r d", r=JT)
            for g4 in range(0, JT, 4):
                nc.gpsimd.dma_start(kvs[:, g4 : g4 + 4], kvv[:, g4 : g4 + 4])
            q_sb = qkv.tile([P, I], BF16, tag="q_sb", name=f"q_sb_{k}")
            qv = q_d[bh].rearrange("(p r) d -> p r d", r=IT)
            qs = q_sb[:].rearrange("p (r d) -> p r d", r=IT)
            for g4 in range(0, IT, 4):
                nc.gpsimd.dma_start(qs[:, g4 : g4 + 4], qv[:, g4 : g4 + 4])
            return q_sb, kv_sb

        def _setup_steps(k, q_sb, kv_sb):
            """Closures: kvT/qT transposes, vproj (+bias), uT — all bf16."""
            T = TILES[k] = {}
            T["qT"] = tp.tile([P, I], BF16, tag="qT", name=f"qT_{k}")
            T["kvT"] = tp.tile([P, J], BF16, tag="kvT", name=f"kvT_{k}")
            T["uT"] = tp.tile([P, I], BF16, tag="uT", name=f"uT_{k}")
            T["vproj"] = tp.tile([P, J], BF16, tag="vproj", name=f"vp_{k}")
            steps = []

            def tr_group(dst, src, g4):
                pt = ps_sx.tile([P, IC], BF16, tag="sx", name=f"pt_{k}_{g4}")
                for t in range(4):
                    nc.tensor.transpose(
                        pt[:, t * P : (t + 1) * P],
                        src[:, (g4 + t) * P : (g4 + t + 1) * P],
                        ident_bf[:],
                    )
                nc.vector.tensor_copy(dst[:, g4 * P : (g4 + 4) * P], pt[:])

            for dst, src, nt in ((T["kvT"], kv_sb, JT), (T["qT"], q_sb, IT)):
                for g4 in range(0, nt, 4):
                    steps.append(lambda dst=dst, src=src, g4=g4: tr_group(dst, src, g4))

            def vproj_step(g4):
                # 4 jt-blocks of vproj into one scratch bank, one evac+bias
                pvp = ps_sx.tile([P, IC], F32, tag="sx", name=f"pvp_{k}_{g4}")
                for t in range(4):
                    nc.tensor.matmul(
                        pvp[:, t * P : (t + 1) * P],
                        T["kvT"][:, (g4 + t) * P : (g4 + t + 1) * P],
                        wvo_bf[:],
                        start=True, stop=True,
                    )
                nc.vector.tensor_add(
                    T["vproj"][:, g4 * P : (g4 + 4) * P], pvp[:], bout_b[:]
                )

            for g4 in range(0, JT, 4):
                steps.append(lambda g4=g4: vproj_step(g4))

            def ut_step(c):
                pu = ps_sx.tile([P, IC], F32, tag="sx", name=f"pu_{k}_{c}")
                nc.tensor.matmul(
                    pu[:], a_bf[:], T["qT"][:, c * IC : (c + 1) * IC],
                    start=True, stop=True,
                )
                nc.vector.tensor_copy(T["uT"][:, c * IC : (c + 1) * IC], pu[:])

            for c in range(I // IC):
                steps.append(lambda c=c: ut_step(c))
            return steps

        def _finalize(k, h, pv_ps, dn_ps, out_sb):
            pvT = []
            for c in range(2):
                t_ = fin.tile([P, IC], BF16, tag="pvT", name=f"pvT_{k}_{h}_{c}")
                nc.vector.tensor_copy(t_[:], pv_ps[c][:])
                pvT.append(t_)
            dn_sb = fin.tile([P, IC], F32, tag="dn_sb", name=f"dnsb_{k}_{h}")
            nc.vector.tensor_copy(dn_sb[0:33], dn_ps[0:33])

            # pdt/recip FIRST: the p_o transposes' readers (the muls below)
            # need recip; with a single-slot scratch pool, allocating po
            # before pdt makes pdt's slot wait on po's readers -> deadlock.
            # tiny matmuls transpose the 8 dn row-segments into [i-part, 8];
            # lhsT is the dn row segment (base partition 32c), so the rhs
            # ones slice must share that base partition.
            pdt = ps_sx.tile([P, IC], F32, tag="sx", name=f"pdt_{k}_{h}")
            for c in range(2):
                for t in range(4):
                    nc.tensor.matmul(
                        pdt[:, 4 * c + t : 4 * c + t + 1],
                        dn_sb[32 * c : 32 * c + 1, t * P : (t + 1) * P],
                        ones_f32[32 * c : 32 * c + 1, 0:1],
                        start=True, stop=True,
                    )
            recip = fin.tile([P, 8], F32, tag="recip", name=f"rc_{k}_{h}")
            nc.vector.reciprocal(recip[:], pdt[:, 0:8])

            # transpose pv chunks (i-tile blocks) via PE; 4 per scratch bank
            for c in range(2):
                po = ps_sx.tile([P, IC], BF16, tag="sx", name=f"po_{k}_{h}_{c}")
                for t in range(4):
                    nc.tensor.transpose(
                        po[:, t * P : (t + 1) * P],
                        pvT[c][:, t * P : (t + 1) * P],
                        ident_bf[:],
                    )
                for t in range(4):
                    tg = 8 * h + 4 * c + t       # global tile: i = 16p + tg
                    nc.vector.tensor_scalar_mul(
                        out_sb[:, tg * P : (tg + 1) * P],
                        po[:, t * P : (t + 1) * P],
                        recip[:, 4 * c + t : 4 * c + t + 1],
                    )

        def _main(k, interleave):
            bh = tasks[k][1]
            T = TILES[k]
            kvT, uT, vproj = T["kvT"], T["uT"], T["vproj"]
            out_sb = fin.tile([P, I], F32, tag="out_sb", name=f"out_sb_{k}")
            for h in range(2):
                # dn accumulation: DVE-memset the rows to 0, then every
                # matmul runs start=False. Whatever the has_written state,
                # the result is correct: bits set -> accumulate onto 0;
                # bits unset -> overwrite with the fresh contribution.
                dn_ps = ps_dn.tile([P, IC], F32, tag="dn", name=f"dn_{k}_{h}")
                nc.vector.memset(dn_ps[0:33], 0.0)
                pv_ps = [
                    ps_pv.tile([P, IC], F32, tag="pv", name=f"pv_{k}_{h}_{c}")
                    for c in range(2)
                ]
                for jt in range(JT):
                    sc = ps_sc.tile([P, IH], F32, tag="sc", name=f"sc_{k}_{h}_{jt}")
                    for c in range(2):
                        nc.tensor.matmul(
                            sc[:, c * IC : (c + 1) * IC],
                            kvT[:, jt * P : (jt + 1) * P],
                            uT[:, h * IH + c * IC : h * IH + (c + 1) * IC],
                            start=True, stop=True,
                        )
                    e_sb = ep.tile([P, IH], BF16, tag="e_sb", name=f"e_{k}_{h}_{jt}")
                    # two FD=512 ACTIVATEs, each within one PSUM bank
                    for c in range(2):
                        nc.scalar.activation(
                            e_sb[:, c * IC : (c + 1) * IC],
                            sc[:, c * IC : (c + 1) * IC],
                            EXP, scale=SCALE,
                        )
                    for c in range(2):
                        nc.tensor.matmul(
                            pv_ps[c][:],
                            vproj[:, jt * P : (jt + 1) * P],
                            e_sb[:, c * IC : (c + 1) * IC],
                            start=(jt == 0), stop=(jt == JT - 1),
                        )
                    for c in range(2):
                        nc.tensor.matmul(
                            dn_ps[32 * c : 32 * c + 1, :],
                            ones_bf[:],
                            e_sb[:, c * IC : (c + 1) * IC],
                            start=False,
                            stop=(jt == JT - 1 and c == 1),
                            tile_position=(0, 32 * c),
                            skip_group_check=True,
                        )
                    if interleave:
                        interleave.pop(0)()
                _finalize(k, h, pv_ps, dn_ps, out_sb)
            for s in interleave:
                s()
            # single 1MB store: partition p holds rows 16p..16p+15
            nc.sync.dma_start(
                out_d[bh].rearrange("(p r) e -> p r e", r=IT),
                out_sb[:].rearrange("p (r e) -> p r e", r=IT),
            )
            del TILES[k]

        # prologue: task 0 loads + full setup
        q0 = _loads(0)
        for s in _setup_steps(0, *q0):
            s()
        for k in range(len(tasks)):
            pending = []
            if k + 1 < len(tasks):
                qn = _loads(k + 1)
                pending = _setup_steps(k + 1, *qn)
            _main(k, pending)

    nc.compile()
    return nc


def kernel(q, kv, Wq, Wkv, Wout, bout):
    if "nc" not in _cache:
        _cache["nc"] = _build()
    nc = _cache["nc"]

    Wk = Wkv[:D].astype(np.float64)
    Wv = Wkv[D:].astype(np.float64)
    A = (Wq.astype(np.float64).T @ Wk).astype(np.float32)
    WvoT = (Wv.T @ Wout.astype(np.float64).T).astype(np.float32)
    bout_b = np.tile(np.asarray(bout, np.float32)[None, :], (P, 4)).copy()
    ident = np.eye(P, dtype=np.float32)

    qf = np.ascontiguousarray(np.asarray(q, np.float32).reshape(BH, I, D))
    kvf = np.ascontiguousarray(np.asarray(kv, np.float32).reshape(BH, J, D))

    in_maps = []
    for c in range(N_CORES):
        sl = slice(c * PER_CORE, (c + 1) * PER_CORE)
        in_maps.append(
            {
                "q": np.ascontiguousarray(qf[sl]),
                "kv": np.ascontiguousarray(kvf[sl]),
                "A": A,
                "WvoT": WvoT,
                "bout_b": bout_b,
                "ident": ident,
            }
        )

    global _last_in_maps
    _last_in_maps = in_maps

    res = run_bass_kernel_spmd(nc, in_maps, core_ids=list(range(N_CORES)))
    out = np.concatenate([r["out"] for r in res.results], axis=0)
    return out.reshape(B, H, I, D)


_last_in_maps = None


# revision 16
# speedup vs baseline: 1.2000x; 1.2000x over previous
"""Trainium2 Bass kernel for nn_CrossAtten: cross-attention
out = softmax((q Wq^T)(kv Wk^T)^T / sqrt(D)) @ (kv Wv^T) @ Wout^T + bout

Shapes (hardcoded): q,kv [4,16,2048,128] fp32; Wq,Wout [128,128]; Wkv [256,128]; bout [128].
Sharding: batch*heads (64 pairs) split 8 per NeuronCore across 8 cores (pure data parallel).

v2 design (bf16 pipeline, jt-major, packed denominator):
  A    = Wq^T @ Wk          -> scores S^T = (A^T q^T)^T kv^T  (u = qA)
  WvoT = Wv^T @ Wout^T      -> PV matmul directly yields final projection
  bout folded into vproj (vproj += bout) so pv accumulates pv + bout*dn and
  the final divide by dn yields out + bout directly.
  All PE operands bf16 (1 cyc/row, FWL weight loads); PSUM accum stays fp32.
  Main loop is jt-major over i-halves (1024 i per half): the kvT[jt]/vproj[jt]
  stationaries are shared across the half's 2 chunks, exp runs as one
  FD=1024 ACTIVATE per (half, jt), and the two denominator matmuls (M=1)
  are packed into one PSUM bank at col strips 0/32 via tile_position so they
  stream concurrently. dn accumulates with start=False onto a memset-zeroed
  bank (DVE memset leaves has_written bits, so matmuls accumulate onto 0).
"""
import sys

if "/opt/trn_rl_repo" not in sys.path:
    sys.path.insert(0, "/opt/trn_rl_repo")

from contextlib import ExitStack

import numpy as np

import concourse.bacc as bacc
import concourse.tile as tile
import concourse.mybir as mybir
from concourse.bass_utils import run_bass_kernel_spmd

B, H, I, J, D = 4, 16, 2048, 2048, 128
BH = B * H
N_CORES = 8
PER_CORE = BH // N_CORES          # 8 (b,h) pairs per core
P = 128                           # partitions
IT = I // P                       # 16 i-tiles
JT = J // P                       # 16 j-tiles
IH = 1024                         # i-half width
IC = 512                          # i-chunk (columns per scores/PV matmul)
SCALE = D ** -0.5

F32 = mybir.dt.float32
BF16 = mybir.dt.bfloat16
EXP = mybir.ActivationFunctionType.Exp

_cache = {}


def _build(repeat=1):
    nc = bacc.Bacc(
        "TRN2",
        target_bir_lowering=False,
        debug=False,
        enable_asserts=False,
        num_devices=N_CORES,
    )

    q_d = nc.dram_tensor("q", [PER_CORE, I, D], F32, kind="ExternalInput").ap()
    kv_d = nc.dram_tensor("kv", [PER_CORE, J, D], F32, kind="ExternalInput").ap()
    a_d = nc.dram_tensor("A", [D, D], F32, kind="ExternalInput").ap()
    wvo_d = nc.dram_tensor("WvoT", [D, D], F32, kind="ExternalInput").ap()
    boutb_d = nc.dram_tensor("bout_b", [P, 4 * D], F32, kind="ExternalInput").ap()
    ident_d = nc.dram_tensor("ident", [P, P], F32, kind="ExternalInput").ap()
    out_d = nc.dram_tensor("out", [PER_CORE, I, D], F32, kind="ExternalOutput").ap()

    with tile.TileContext(nc) as tc, ExitStack() as ctx:
        const = ctx.enter_context(tc.tile_pool(name="const", bufs=1))
        qkv = ctx.enter_context(tc.tile_pool(name="qkv", bufs=2))
        tp = ctx.enter_context(tc.tile_pool(name="tp", bufs=2))
        ep = ctx.enter_context(tc.tile_pool(name="ep", bufs=4))
        fin = ctx.enter_context(tc.tile_pool(name="fin", bufs=2))
        # PSUM: sc 2x[128,1024]f32 (4 banks) + pv 2x[128,512]f32 (2) +
        # dn 1x[128,512]f32 (1) + scratch 1x[128,512]f32 (1) = 8 banks
        ps_sc = ctx.enter_context(tc.tile_pool(name="ps_sc", bufs=2, space="PSUM"))
        ps_pv = ctx.enter_context(tc.tile_pool(name="ps_pv", bufs=2, space="PSUM"))
        ps_dn = ctx.enter_context(tc.tile_pool(name="ps_dn", bufs=1, space="PSUM"))
        ps_sx = ctx.enter_context(tc.tile_pool(name="ps_sx", bufs=1, space="PSUM"))

        # ---- constants (loaded / prepared once, cast to bf16 in DMA) ----
        ident_bf = const.tile([P, P], BF16, tag="ident_bf")
        nc.gpsimd.dma_start(ident_bf[:], ident_d)
        a_bf = const.tile([D, D], BF16, tag="a_bf")
        nc.gpsimd.dma_start(a_bf[:], a_d)
        wvo_bf = const.tile([D, D], BF16, tag="wvo_bf")
        nc.gpsimd.dma_start(wvo_bf[:], wvo_d)
        # bias broadcast over j-partitions, tiled 4x along free dim so a
        # 4-jt vproj evacuation adds it in one tensor_tensor
        bout_b = const.tile([P, 4 * D], F32, tag="bout_b")
        nc.sync.dma_start(bout_b[:], boutb_d)
        ones_bf = const.tile([P, 1], BF16, tag="ones_bf")
        nc.vector.memset(ones_bf[:], 1.0)
        ones_f32 = const.tile([P, 1], F32, tag="ones_f32")
        nc.vector.memset(ones_f32[:], 1.0)

        tasks = [(r, b) for r in range(repeat) for b in range(PER_CORE)]
        TILES = {}

        def _loads(k):
            bh = tasks[k][1]
            # partition p holds rows 16p..16p+15 (i = 16p + r): contiguous
            # 2KB fp32 per partition per 4-row group; SWDGE casts to bf16.
            kv_sb = qkv.tile([P, J], BF16, tag="kv_sb", name=f"kv_sb_{k}")
            kvv = kv_d[bh].rearrange("(p r) d -> p r d", r=JT)
            kvs = kv_sb[:].rearrange("p (r d) -> p r d", r=JT)
            for g4 in range(0, JT, 4):
                nc.gpsimd.dma_start(kvs[:, g4 : g4 + 4], kvv[:, g4 : g4 + 4])
            q_sb = qkv.tile([P, I], BF16, tag="q_sb", name=f"q_sb_{k}")
            qv = q_d[bh].rearrange("(p r) d -> p r d", r=IT)
            qs = q_sb[:].rearrange("p (r d) -> p r d", r=IT)
            for g4 in range(0, IT, 4):
                nc.gpsimd.dma_start(qs[:, g4 : g4 + 4], qv[:, g4 : g4 + 4])
            return q_sb, kv_sb

        def _setup_steps(k, q_sb, kv_sb):
            """Closures: kvT/qT transposes, vproj (+bias), uT — all bf16."""
            T = TILES[k] = {}
            T["qT"] = tp.tile([P, I], BF16, tag="qT", name=f"qT_{k}")
            T["kvT"] = tp.tile([P, J], BF16, tag="kvT", name=f"kvT_{k}")
            T["uT"] = tp.tile([P, I], BF16, tag="uT", name=f"uT_{k}")
            T["vproj"] = tp.tile([P, J], BF16, tag="vproj", name=f"vp_{k}")
            steps = []

            def tr_group(dst, src, g4):
                pt = ps_sx.tile([P, IC], BF16, tag="sx", name=f"pt_{k}_{g4}")
                for t in range(4):
                    nc.tensor.transpose(
                        pt[:, t * P : (t + 1) * P],
                        src[:, (g4 + t) * P : (g4 + t + 1) * P],
                        ident_bf[:],
                    )
                nc.vector.tensor_copy(dst[:, g4 * P : (g4 + 4) * P], pt[:])

            for dst, src, nt in ((T["kvT"], kv_sb, JT), (T["qT"], q_sb, IT)):
                for g4 in range(0, nt, 4):
                    steps.append(lambda dst=dst, src=src, g4=g4: tr_group(dst, src, g4))

            def vproj_step(g4):
                # 4 jt-blocks of vproj into one scratch bank, one evac+bias
                pvp = ps_sx.tile([P, IC], F32, tag="sx", name=f"pvp_{k}_{g4}")
                for t in range(4):
                    nc.tensor.matmul(
                        pvp[:, t * P : (t + 1) * P],
                        T["kvT"][:, (g4 + t) * P : (g4 + t + 1) * P],
                        wvo_bf[:],
                        start=True, stop=True,
                    )
                nc.vector.tensor_add(
                    T["vproj"][:, g4 * P : (g4 + 4) * P], pvp[:], bout_b[:]
                )

            for g4 in range(0, JT, 4):
                steps.append(lambda g4=g4: vproj_step(g4))

            def ut_step(c):
                pu = ps_sx.tile([P, IC], F32, tag="sx", name=f"pu_{k}_{c}")
                nc.tensor.matmul(
                    pu[:], a_bf[:], T["qT"][:, c * IC : (c + 1) * IC],
                    start=True, stop=True,
                )
                nc.vector.tensor_copy(T["uT"][:, c * IC : (c + 1) * IC], pu[:])

            for c in range(I // IC):
                steps.append(lambda c=c: ut_step(c))
            return steps

        def _finalize(k, h, pv_ps, dn_ps, out_sb):
            pvT = []
            for c in range(2):
                t_ = fin.tile([P, IC], BF16, tag="pvT", name=f"pvT_{k}_{h}_{c}")
                nc.vector.tensor_copy(t_[:], pv_ps[c][:])
                pvT.append(t_)
            dn_sb = fin.tile([P, IC], F32, tag="dn_sb", name=f"dnsb_{k}_{h}")
            nc.vector.tensor_copy(dn_sb[0:33], dn_ps[0:33])

            # pdt/recip FIRST: the p_o transposes' readers (the muls below)
            # need recip; with a single-slot scratch pool, allocating po
            # before pdt makes pdt's slot wait on po's readers -> deadlock.
            # tiny matmuls transpose the 8 dn row-segments into [i-part, 8];
            # lhsT is the dn row segment (base partition 32c), so the rhs
            # ones slice must share that base partition.
            pdt = ps_sx.tile([P, IC], F32, tag="sx", name=f"pdt_{k}_{h}")
            for c in range(2):
                for t in range(4):
                    nc.tensor.matmul(
                        pdt[:, 4 * c + t : 4 * c + t + 1],
                        dn_sb[32 * c : 32 * c + 1, t * P : (t + 1) * P],
                        ones_f32[32 * c : 32 * c + 1, 0:1],
                        start=True, stop=True,
                    )
            recip = fin.tile([P, 8], F32, tag="recip", name=f"rc_{k}_{h}")
            nc.vector.reciprocal(recip[:], pdt[:, 0:8])

            # transpose pv chunks (i-tile blocks) via PE; 4 per scratch bank
            for c in range(2):
                po = ps_sx.tile([P, IC], BF16, tag="sx", name=f"po_{k}_{h}_{c}")
                for t in range(4):
                    nc.tensor.transpose(
                        po[:, t * P : (t + 1) * P],
                        pvT[c][:, t * P : (t + 1) * P],
                        ident_bf[:],
                    )
                for t in range(4):
                    tg = 8 * h + 4 * c + t       # global tile: i = 16p + tg
                    nc.vector.tensor_scalar_mul(
                        out_sb[:, tg * P : (tg + 1) * P],
                        po[:, t * P : (t + 1) * P],
                        recip[:, 4 * c + t : 4 * c + t + 1],
                    )

        def _main(k, interleave):
            bh = tasks[k][1]
            T = TILES[k]
            kvT, uT, vproj = T["kvT"], T["uT"], T["vproj"]
            out_sb = fin.tile([P, I], F32, tag="out_sb", name=f"out_sb_{k}")
            for h in range(2):
                # dn accumulation: DVE-memset the rows to 0, then every
                # matmul runs start=False. Whatever the has_written state,
                # the result is correct: bits set -> accumulate onto 0;
                # bits unset -> overwrite with the fresh contribution.
                dn_ps = ps_dn.tile([P, IC], F32, tag="dn", name=f"dn_{k}_{h}")
                nc.vector.memset(dn_ps[0:33], 0.0)
                pv_ps = [
                    ps_pv.tile([P, IC], F32, tag="pv", name=f"pv_{k}_{h}_{c}")
                    for c in range(2)
                ]
                # software-pipelined: scores/exp for step g are emitted
                # alongside pv/dn for step g-SK, so no PE instruction at the
                # queue head is waiting on a just-issued ACT result (the PE
                # queue is in-order; a waiting head blocks everything).
                SK = 2
                E = {}
                for g in range(JT + SK):
                    if g < JT:
                        sc = ps_sc.tile(
                            [P, IH], F32, tag="sc", name=f"sc_{k}_{h}_{g}"
                        )
                        for c in range(2):
                            nc.tensor.matmul(
                                sc[:, c * IC : (c + 1) * IC],
                                kvT[:, g * P : (g + 1) * P],
                                uT[:, h * IH + c * IC : h * IH + (c + 1) * IC],
                                start=True, stop=True,
                            )
                        e_sb = ep.tile(
                            [P, IH], BF16, tag="e_sb", name=f"e_{k}_{h}_{g}"
                        )
                        # two FD=512 ACTIVATEs, each within one PSUM bank
                        for c in range(2):
                            nc.scalar.activation(
                                e_sb[:, c * IC : (c + 1) * IC],
                                sc[:, c * IC : (c + 1) * IC],
                                EXP, scale=SCALE,
                            )
                        E[g] = e_sb
                    if g >= SK:
                        jt = g - SK
                        e_sb = E.pop(jt)
                        for c in range(2):
                            nc.tensor.matmul(
                                pv_ps[c][:],
                                vproj[:, jt * P : (jt + 1) * P],
                                e_sb[:, c * IC : (c + 1) * IC],
                                start=(jt == 0), stop=(jt == JT - 1),
                            )
                        for c in range(2):
                            nc.tensor.matmul(
                                dn_ps[32 * c : 32 * c + 1, :],
                                ones_bf[:],
                                e_sb[:, c * IC : (c + 1) * IC],
                                start=False,
                                stop=(jt == JT - 1 and c == 1),
                                tile_position=(0, 32 * c),
                                skip_group_check=True,
                            )
                    if interleave:
                        interleave.pop(0)()
                _finalize(k, h, pv_ps, dn_ps, out_sb)
            for s in interleave:
                s()
            # single 1MB store: partition p holds rows 16p..16p+15
            nc.sync.dma_start(
                out_d[bh].rearrange("(p r) e -> p r e", r=IT),
                out_sb[:].rearrange("p (r e) -> p r e", r=IT),
            )
            del TILES[k]

        # prologue: task 0 loads + full setup
        q0 = _loads(0)
        for s in _setup_steps(0, *q0):
            s()
        for k in range(len(tasks)):
            pending = []
            if k + 1 < len(tasks):
                qn = _loads(k + 1)
                pending = _setup_steps(k + 1, *qn)
            _main(k, pending)

    nc.compile()
    return nc


def kernel(q, kv, Wq, Wkv, Wout, bout):
    if "nc" not in _cache:
        _cache["nc"] = _build()
    nc = _cache["nc"]

    Wk = Wkv[:D].astype(np.float64)
    Wv = Wkv[D:].astype(np.float64)
    A = (Wq.astype(np.float64).T @ Wk).astype(np.float32)
    WvoT = (Wv.T @ Wout.astype(np.float64).T).astype(np.float32)
    bout_b = np.tile(np.asarray(bout, np.float32)[None, :], (P, 4)).copy()
    ident = np.eye(P, dtype=np.float32)

    qf = np.ascontiguousarray(np.asarray(q, np.float32).reshape(BH, I, D))
    kvf = np.ascontiguousarray(np.asarray(kv, np.float32).reshape(BH, J, D))

    in_maps = []
    for c in range(N_CORES):
        sl = slice(c * PER_CORE, (c + 1) * PER_CORE)
        in_maps.append(
            {
                "q": np.ascontiguousarray(qf[sl]),
                "kv": np.ascontiguousarray(kvf[sl]),
                "A": A,
                "WvoT": WvoT,
                "bout_b": bout_b,
                "ident": ident,
            }
        )

    global _last_in_maps
    _last_in_maps = in_maps

    res = run_bass_kernel_spmd(nc, in_maps, core_ids=list(range(N_CORES)))
    out = np.concatenate([r["out"] for r in res.results], axis=0)
    return out.reshape(B, H, I, D)


_last_in_maps = None


# revision 17
# speedup vs baseline: 1.2674x; 1.0562x over previous
"""Trainium2 Bass kernel for nn_CrossAtten: cross-attention
out = softmax((q Wq^T)(kv Wk^T)^T / sqrt(D)) @ (kv Wv^T) @ Wout^T + bout

Shapes (hardcoded): q,kv [4,16,2048,128] fp32; Wq,Wout [128,128]; Wkv [256,128]; bout [128].
Sharding: batch*heads (64 pairs) split 8 per NeuronCore across 8 cores (pure data parallel).

v2 design (bf16 pipeline, jt-major, packed denominator):
  A    = Wq^T @ Wk          -> scores S^T = (A^T q^T)^T kv^T  (u = qA)
  WvoT = Wv^T @ Wout^T      -> PV matmul directly yields final projection
  bout folded into vproj (vproj += bout) so pv accumulates pv + bout*dn and
  the final divide by dn yields out + bout directly.
  All PE operands bf16 (1 cyc/row, FWL weight loads); PSUM accum stays fp32.
  Main loop is jt-major over i-halves (1024 i per half): the kvT[jt]/vproj[jt]
  stationaries are shared across the half's 2 chunks, exp runs as one
  FD=1024 ACTIVATE per (half, jt), and the two denominator matmuls (M=1)
  are packed into one PSUM bank at col strips 0/32 via tile_position so they
  stream concurrently. dn accumulates with start=False onto a memset-zeroed
  bank (DVE memset leaves has_written bits, so matmuls accumulate onto 0).
"""
import sys

if "/opt/trn_rl_repo" not in sys.path:
    sys.path.insert(0, "/opt/trn_rl_repo")

from contextlib import ExitStack

import numpy as np

import concourse.bacc as bacc
import concourse.tile as tile
import concourse.mybir as mybir
from concourse.bass_utils import run_bass_kernel_spmd

B, H, I, J, D = 4, 16, 2048, 2048, 128
BH = B * H
N_CORES = 8
PER_CORE = BH // N_CORES          # 8 (b,h) pairs per core
P = 128                           # partitions
IT = I // P                       # 16 i-tiles
JT = J // P                       # 16 j-tiles
IH = 1024                         # i-half width
IC = 512                          # i-chunk (columns per scores/PV matmul)
SCALE = D ** -0.5

F32 = mybir.dt.float32
BF16 = mybir.dt.bfloat16
EXP = mybir.ActivationFunctionType.Exp

_cache = {}


def _build(repeat=1):
    nc = bacc.Bacc(
        "TRN2",
        target_bir_lowering=False,
        debug=False,
        enable_asserts=False,
        num_devices=N_CORES,
    )

    q_d = nc.dram_tensor("q", [PER_CORE, I, D], F32, kind="ExternalInput").ap()
    kv_d = nc.dram_tensor("kv", [PER_CORE, J, D], F32, kind="ExternalInput").ap()
    a_d = nc.dram_tensor("A", [D, D], F32, kind="ExternalInput").ap()
    wvo_d = nc.dram_tensor("WvoT", [D, D], F32, kind="ExternalInput").ap()
    boutb_d = nc.dram_tensor("bout_b", [P, 4 * D], F32, kind="ExternalInput").ap()
    ident_d = nc.dram_tensor("ident", [P, P], F32, kind="ExternalInput").ap()
    out_d = nc.dram_tensor("out", [PER_CORE, I, D], F32, kind="ExternalOutput").ap()

    with tile.TileContext(nc) as tc, ExitStack() as ctx:
        const = ctx.enter_context(tc.tile_pool(name="const", bufs=1))
        qkv = ctx.enter_context(tc.tile_pool(name="qkv", bufs=2))
        tp = ctx.enter_context(tc.tile_pool(name="tp", bufs=2))
        ep = ctx.enter_context(tc.tile_pool(name="ep", bufs=4))
        fin = ctx.enter_context(tc.tile_pool(name="fin", bufs=2))
        # PSUM: sc 2x[128,1024]f32 (4 banks) + pv 2x[128,512]f32 (2) +
        # dn 1x[128,512]f32 (1) + scratch 1x[128,512]f32 (1) = 8 banks
        ps_sc = ctx.enter_context(tc.tile_pool(name="ps_sc", bufs=2, space="PSUM"))
        ps_pv = ctx.enter_context(tc.tile_pool(name="ps_pv", bufs=2, space="PSUM"))
        ps_dn = ctx.enter_context(tc.tile_pool(name="ps_dn", bufs=1, space="PSUM"))
        ps_sx = ctx.enter_context(tc.tile_pool(name="ps_sx", bufs=1, space="PSUM"))

        # ---- constants (loaded / prepared once, cast to bf16 in DMA) ----
        ident_bf = const.tile([P, P], BF16, tag="ident_bf")
        nc.gpsimd.dma_start(ident_bf[:], ident_d)
        a_bf = const.tile([D, D], BF16, tag="a_bf")
        nc.gpsimd.dma_start(a_bf[:], a_d)
        wvo_bf = const.tile([D, D], BF16, tag="wvo_bf")
        nc.gpsimd.dma_start(wvo_bf[:], wvo_d)
        # bias broadcast over j-partitions, tiled 4x along free dim so a
        # 4-jt vproj evacuation adds it in one tensor_tensor
        bout_b = const.tile([P, 4 * D], F32, tag="bout_b")
        nc.sync.dma_start(bout_b[:], boutb_d)
        ones_bf = const.tile([P, 1], BF16, tag="ones_bf")
        nc.vector.memset(ones_bf[:], 1.0)
        ones_f32 = const.tile([P, 1], F32, tag="ones_f32")
        nc.vector.memset(ones_f32[:], 1.0)

        tasks = [(r, b) for r in range(repeat) for b in range(PER_CORE)]
        TILES = {}

        def _loads(k):
            bh = tasks[k][1]
            # partition p holds rows 16p..16p+15 (i = 16p + r): contiguous
            # 2KB fp32 per partition per 4-row group; SWDGE casts to bf16.
            kv_sb = qkv.tile([P, J], BF16, tag="kv_sb", name=f"kv_sb_{k}")
            kvv = kv_d[bh].rearrange("(p r) d -> p r d", r=JT)
            kvs = kv_sb[:].rearrange("p (r d) -> p r d", r=JT)
            for g4 in range(0, JT, 4):
                nc.gpsimd.dma_start(kvs[:, g4 : g4 + 4], kvv[:, g4 : g4 + 4])
            q_sb = qkv.tile([P, I], BF16, tag="q_sb", name=f"q_sb_{k}")
            qv = q_d[bh].rearrange("(p r) d -> p r d", r=IT)
            qs = q_sb[:].rearrange("p (r d) -> p r d", r=IT)
            for g4 in range(0, IT, 4):
                nc.gpsimd.dma_start(qs[:, g4 : g4 + 4], qv[:, g4 : g4 + 4])
            return q_sb, kv_sb

        def _setup_steps(k, q_sb, kv_sb):
            """Closures: kvT/qT transposes, vproj (+bias), uT — all bf16."""
            T = TILES[k] = {}
            T["qT"] = tp.tile([P, I], BF16, tag="qT", name=f"qT_{k}")
            T["kvT"] = tp.tile([P, J], BF16, tag="kvT", name=f"kvT_{k}")
            T["uT"] = tp.tile([P, I], BF16, tag="uT", name=f"uT_{k}")
            T["vproj"] = tp.tile([P, J], BF16, tag="vproj", name=f"vp_{k}")
            steps = []

            def tr_group(dst, src, g4):
                pt = ps_sx.tile([P, IC], BF16, tag="sx", name=f"pt_{k}_{g4}")
                for t in range(4):
                    nc.tensor.transpose(
                        pt[:, t * P : (t + 1) * P],
                        src[:, (g4 + t) * P : (g4 + t + 1) * P],
                        ident_bf[:],
                    )
                nc.vector.tensor_copy(dst[:, g4 * P : (g4 + 4) * P], pt[:])

            for dst, src, nt in ((T["kvT"], kv_sb, JT), (T["qT"], q_sb, IT)):
                for g4 in range(0, nt, 4):
                    steps.append(lambda dst=dst, src=src, g4=g4: tr_group(dst, src, g4))

            def vproj_step(g4):
                # 4 jt-blocks of vproj into one scratch bank, one evac+bias
                pvp = ps_sx.tile([P, IC], F32, tag="sx", name=f"pvp_{k}_{g4}")
                for t in range(4):
                    nc.tensor.matmul(
                        pvp[:, t * P : (t + 1) * P],
                        T["kvT"][:, (g4 + t) * P : (g4 + t + 1) * P],
                        wvo_bf[:],
                        start=True, stop=True,
                    )
                nc.vector.tensor_add(
                    T["vproj"][:, g4 * P : (g4 + 4) * P], pvp[:], bout_b[:]
                )

            for g4 in range(0, JT, 4):
                steps.append(lambda g4=g4: vproj_step(g4))

            def ut_step(c):
                pu = ps_sx.tile([P, IC], F32, tag="sx", name=f"pu_{k}_{c}")
                nc.tensor.matmul(
                    pu[:], a_bf[:], T["qT"][:, c * IC : (c + 1) * IC],
                    start=True, stop=True,
                )
                nc.vector.tensor_copy(T["uT"][:, c * IC : (c + 1) * IC], pu[:])

            for c in range(I // IC):
                steps.append(lambda c=c: ut_step(c))
            return steps

        def _finalize(k, h, pv_ps, dn_ps, out_sb):
            pvT = []
            for c in range(2):
                t_ = fin.tile([P, IC], BF16, tag="pvT", name=f"pvT_{k}_{h}_{c}")
                nc.vector.tensor_copy(t_[:], pv_ps[c][:])
                pvT.append(t_)
            dn_sb = fin.tile([P, IC], F32, tag="dn_sb", name=f"dnsb_{k}_{h}")
            nc.vector.tensor_copy(dn_sb[0:33], dn_ps[0:33])

            # pdt/recip FIRST: the p_o transposes' readers (the muls below)
            # need recip; with a single-slot scratch pool, allocating po
            # before pdt makes pdt's slot wait on po's readers -> deadlock.
            # tiny matmuls transpose the 8 dn row-segments into [i-part, 8];
            # lhsT is the dn row segment (base partition 32c), so the rhs
            # ones slice must share that base partition.
            pdt = ps_sx.tile([P, IC], F32, tag="sx", name=f"pdt_{k}_{h}")
            for c in range(2):
                for t in range(4):
                    nc.tensor.matmul(
                        pdt[:, 4 * c + t : 4 * c + t + 1],
                        dn_sb[32 * c : 32 * c + 1, t * P : (t + 1) * P],
                        ones_f32[32 * c : 32 * c + 1, 0:1],
                        start=True, stop=True,
                    )
            recip = fin.tile([P, 8], F32, tag="recip", name=f"rc_{k}_{h}")
            nc.vector.reciprocal(recip[:], pdt[:, 0:8])

            # transpose pv chunks (i-tile blocks) via PE; 4 per scratch bank
            for c in range(2):
                po = ps_sx.tile([P, IC], BF16, tag="sx", name=f"po_{k}_{h}_{c}")
                for t in range(4):
                    nc.tensor.transpose(
                        po[:, t * P : (t + 1) * P],
                        pvT[c][:, t * P : (t + 1) * P],
                        ident_bf[:],
                    )
                for t in range(4):
                    tg = 8 * h + 4 * c + t       # global tile: i = 16p + tg
                    nc.vector.tensor_scalar_mul(
                        out_sb[:, tg * P : (tg + 1) * P],
                        po[:, t * P : (t + 1) * P],
                        recip[:, 4 * c + t : 4 * c + t + 1],
                    )

        def _main(k, interleave):
            bh = tasks[k][1]
            T = TILES[k]
            kvT, uT, vproj = T["kvT"], T["uT"], T["vproj"]
            out_sb = fin.tile([P, I], F32, tag="out_sb", name=f"out_sb_{k}")
            for h in range(2):
                # dn accumulation: DVE-memset the rows to 0, then every
                # matmul runs start=False. Whatever the has_written state,
                # the result is correct: bits set -> accumulate onto 0;
                # bits unset -> overwrite with the fresh contribution.
                dn_ps = ps_dn.tile([P, IC], F32, tag="dn", name=f"dn_{k}_{h}")
                nc.vector.memset(dn_ps[0:33], 0.0)
                pv_ps = [
                    ps_pv.tile([P, IC], F32, tag="pv", name=f"pv_{k}_{h}_{c}")
                    for c in range(2)
                ]
                # software-pipelined: scores/exp for step g are emitted
                # alongside pv/dn for step g-SK, so no PE instruction at the
                # queue head is waiting on a just-issued ACT result (the PE
                # queue is in-order; a waiting head blocks everything).
                SK = 2
                E = {}
                for g in range(JT + SK):
                    if g < JT:
                        sc = ps_sc.tile(
                            [P, IH], F32, tag="sc", name=f"sc_{k}_{h}_{g}"
                        )
                        for c in range(2):
                            nc.tensor.matmul(
                                sc[:, c * IC : (c + 1) * IC],
                                kvT[:, g * P : (g + 1) * P],
                                uT[:, h * IH + c * IC : h * IH + (c + 1) * IC],
                                start=True, stop=True,
                            )
                        e_sb = ep.tile(
                            [P, IH], BF16, tag="e_sb", name=f"e_{k}_{h}_{g}"
                        )
                        # one FD=1024 ACTIVATE spanning both PSUM banks
                        nc.scalar.activation(e_sb[:], sc[:], EXP, scale=SCALE)
                        E[g] = e_sb
                    if g >= SK:
                        jt = g - SK
                        e_sb = E.pop(jt)
                        for c in range(2):
                            nc.tensor.matmul(
                                pv_ps[c][:],
                                vproj[:, jt * P : (jt + 1) * P],
                                e_sb[:, c * IC : (c + 1) * IC],
                                start=(jt == 0), stop=(jt == JT - 1),
                            )
                        for c in range(2):
                            nc.tensor.matmul(
                                dn_ps[32 * c : 32 * c + 1, :],
                                ones_bf[:],
                                e_sb[:, c * IC : (c + 1) * IC],
                                start=False,
                                stop=(jt == JT - 1 and c == 1),
                                tile_position=(0, 32 * c),
                                skip_group_check=True,
                            )
                    if interleave:
                        interleave.pop(0)()
                _finalize(k, h, pv_ps, dn_ps, out_sb)
            for s in interleave:
                s()
            # single 1MB store: partition p holds rows 16p..16p+15
            nc.sync.dma_start(
                out_d[bh].rearrange("(p r) e -> p r e", r=IT),
                out_sb[:].rearrange("p (r e) -> p r e", r=IT),
            )
            del TILES[k]

        # prologue: task 0 loads + full setup
        q0 = _loads(0)
        for s in _setup_steps(0, *q0):
            s()
        for k in range(len(tasks)):
            pending = []
            if k + 1 < len(tasks):
                qn = _loads(k + 1)
                pending = _setup_steps(k + 1, *qn)
            _main(k, pending)

    nc.compile()
    return nc


def kernel(q, kv, Wq, Wkv, Wout, bout):
    if "nc" not in _cache:
        _cache["nc"] = _build()
    nc = _cache["nc"]

    Wk = Wkv[:D].astype(np.float64)
    Wv = Wkv[D:].astype(np.float64)
    A = (Wq.astype(np.float64).T @ Wk).astype(np.float32)
    WvoT = (Wv.T @ Wout.astype(np.float64).T).astype(np.float32)
    bout_b = np.tile(np.asarray(bout, np.float32)[None, :], (P, 4)).copy()
    ident = np.eye(P, dtype=np.float32)

    qf = np.ascontiguousarray(np.asarray(q, np.float32).reshape(BH, I, D))
    kvf = np.ascontiguousarray(np.asarray(kv, np.float32).reshape(BH, J, D))

    in_maps = []
    for c in range(N_CORES):
        sl = slice(c * PER_CORE, (c + 1) * PER_CORE)
        in_maps.append(
            {
                "q": np.ascontiguousarray(qf[sl]),
                "kv": np.ascontiguousarray(kvf[sl]),
                "A": A,
                "WvoT": WvoT,
                "bout_b": bout_b,
                "ident": ident,
            }
        )

    global _last_in_maps
    _last_in_maps = in_maps

    res = run_bass_kernel_spmd(nc, in_maps, core_ids=list(range(N_CORES)))
    out = np.concatenate([r["out"] for r in res.results], axis=0)
    return out.reshape(B, H, I, D)


_last_in_maps = None
